# revision 2
# baseline (speedup 1.0000x reference)
"""Trainium2 Bass kernel v3 for nn_MeshDownConv (2-layer SplineConv GNN).

Same edge-parallel dst-window design as kernel.py, plus:
- bf16 pair-row gathers (table [50176,128] bf16, 256B rows; parity picks the
  64-col half) -> no f32 expand pass, no per-group cast.
- dma_gather batched over 4 groups per instruction (SWDGE emission is ~1us
  fixed per instr).
- dma_scatter_add batched over 4 same-residue groups (disjoint rows).
- za/ohb factorization: DVE builds [x*u, x*u2] plus 3 weighted one-hots
  (oh, oh*v, oh*v2); 6 matmuls/chunk accumulate S[b][node, (a,c)].
- per-batch prefetch of gather-idx and (dstl,u,u2,v,v2) tables from DRAM.
"""
import sys
sys.path.insert(0, '/opt/trn_rl_repo')
import time as _time
import numpy as np
import ml_dtypes

import concourse.bass as bass
import concourse.bacc as bacc
import concourse.mybir as mybir
import concourse.tile as tile
from concourse.masks import make_identity

N_NODES = 100000
N_EDGES = 1600000
NT = 98

BF = ml_dtypes.bfloat16
FP32 = mybir.dt.float32
BF16 = mybir.dt.bfloat16
I16 = mybir.dt.int16
AF = mybir.ActivationFunctionType
ALU = mybir.AluOpType

C = 64
NUM_Q = 9
QDIM = 576                  # 9 * 64 monomial features
N_WCHUNK = 5
N_CORES = 8
WINDOW = 128
N_SUB = 4                   # sub-lists: (parity, seg-half)
KC = 16                     # chunks (128-slot columns) per group
SUB_COLS = KC // N_SUB      # 4 columns per sub-list
SUB_SLOTS = SUB_COLS * 128  # 512 slots per sub-list per group
N_HRAW = 2
import os as _os
GB = int(_os.environ.get('K2_GB', '2'))   # groups per gather batch
SB = int(_os.environ.get('K2_SB', '8'))   # groups per scatter block
K_SC = SB // N_HRAW         # groups per scatter instruction
NMD = 5                     # per-slot table: dstl, u, u2, v, v2

P_BASIS = np.array([[0.5, -1.0, 0.5],
                    [0.5, 1.0, -1.0],
                    [0.0, 0.0, 0.5]], dtype=np.float64)


def reparam_weights(W):
    # monomial-basis W, rows ordered (b, a, c): row = (b*3+a)*C + c
    W33 = np.asarray(W, np.float64).reshape(3, 3, C, C)
    Wp = np.einsum('ia,jb,ijcd->abcd', P_BASIS, P_BASIS, W33)  # [a,b,cin,cout]
    Wp = Wp.transpose(1, 0, 2, 3)                              # [b,a,cin,cout]
    return Wp.reshape(QDIM, C)


def pack_wstack(Wstk):
    # 5 chunks of 128 rows; chunk4 = rows 512:576 + 64 zero rows
    out = np.zeros((128, N_WCHUNK, C), np.float32)
    for j in range(N_WCHUNK):
        w0 = j * 128
        n = min(128, QDIM - w0)
        out[:n, j, :] = Wstk[w0:w0 + n]
    return out.reshape(128, N_WCHUNK * C)


def pack_groups_core(sub, dl, NPC):
    """Greedy-pack dst-sorted edges into <=128-node windows with per-sublist
    512-slot budgets. sub = sub-list key per edge, dl = local dst."""
    node_starts = np.searchsorted(dl, np.arange(NPC + 1))
    groups = []
    n = 0
    while n < NPC:
        if node_starts[n] == node_starts[NPC]:
            break
        w = n
        cnt = np.zeros(N_SUB, np.int64)
        while n < NPC and n - w < WINDOW:
            ea, eb = node_starts[n], node_starts[n + 1]
            if ea == eb:
                n += 1
                continue
            add = np.bincount(sub[ea:eb], minlength=N_SUB)
            if np.any(cnt + add > SUB_SLOTS):
                break
            cnt += add
            n += 1
        assert cnt.sum() > 0, "single node exceeds sub-list budget"
        ea, eb = node_starts[w], node_starts[n]
        idxs = np.arange(ea, eb)
        per_sub = [idxs[sub[ea:eb] == q] for q in range(N_SUB)]
        groups.append((w, per_sub))
    return groups


def prep(edge_index, pseudo, x, W1, root1, b1, W2, root2, b2, n_nodes, NT):
    NPC = NT * 128
    n_src_rows = N_CORES * NPC
    n_pairs = n_src_rows // 2
    PAIR_SEG = n_pairs // 2  # 25088

    src = np.asarray(edge_index[0], np.int64)
    dst = np.asarray(edge_index[1], np.int64)
    order = np.argsort(dst, kind='stable')
    src = src[order]
    dst = dst[order]
    u = np.asarray(pseudo)[order, 0].astype(np.float64)
    v = np.asarray(pseudo)[order, 1].astype(np.float64)

    # sub-list key: parity*2 + seg-half (parity -> which 64-col half)
    par = (src % 2).astype(np.int64)
    seg = ((src // 2) // PAIR_SEG).astype(np.int64)
    sub_all = par * 2 + seg
    pair_idx = (src // 2) - seg * PAIR_SEG  # int16-safe

    core_groups = []
    for c in range(N_CORES):
        lo, hi = c * NPC, min(n_nodes, (c + 1) * NPC)
        a, b = np.searchsorted(dst, lo), np.searchsorted(dst, hi)
        g = pack_groups_core(sub_all[a:b], dst[a:b] - lo, NPC)
        core_groups.append((a, g))
    NG = max(len(g) for _, g in core_groups)
    NG_pad = ((NG + SB - 1) // SB) * SB
    NB = NG_pad // GB
    NS = NG_pad // SB

    in_maps = []
    for c in range(N_CORES):
        a, groups = core_groups[c]
        lo = c * NPC
        # per (group, sublist): 512 idxs (pad with 0) + per-slot md values
        gidx = np.zeros((NG_pad, N_SUB, SUB_SLOTS), np.int16)
        md = np.zeros((NG_pad, KC, 128, NMD), np.float64)
        md[:, :, :, 0] = -1.0  # pad dstl never matches iota
        scat = np.zeros((NG_pad, 128), np.int64)
        wins = np.zeros(NG_pad, np.int64)
        for g in range(NG_pad):
            if g < len(groups):
                w, per_sub = groups[g]
                wins[g] = w
                scat[g] = w + np.arange(128)
                for q in range(N_SUB):
                    es = per_sub[q]  # edge idx into core list
                    k = len(es)
                    gidx[g, q, :k] = pair_idx[a + es]
                    i = np.arange(k)
                    t = q * SUB_COLS + i // 128
                    s = i % 128
                    md[g, t, s, 0] = (dst[a + es] - lo - w).astype(np.float64)
                    md[g, t, s, 1] = u[a + es]
                    md[g, t, s, 2] = u[a + es] ** 2
                    md[g, t, s, 3] = v[a + es]
                    md[g, t, s, 4] = v[a + es] ** 2
            else:
                scat[g] = NPC + np.arange(128)  # junk rows
        # gathw: per (batch, q): 2048 idxs ordered (colq, grp within batch)
        gathw = np.zeros((NB, N_SUB, SUB_COLS, GB, 128), np.int16)
        for b in range(NB):
            for q in range(N_SUB):
                for j in range(GB):
                    gathw[b, q, :, j, :] = gidx[b * GB + j, q].reshape(
                        SUB_COLS, 128)
        gcol = SUB_SLOTS * GB // 16
        gathw = gathw.reshape(NB * N_SUB, SUB_SLOTS * GB)
        gathw = np.ascontiguousarray(
            gathw.reshape(-1, gcol, 16).transpose(0, 2, 1)
        ).reshape(-1, 16, gcol)
        gathw = gathw.transpose(1, 0, 2).reshape(16, -1)
        gathw = np.tile(gathw, (8, 1))
        # scatw: per (block, residue): 4 groups (r, r+3, r+6, r+9) -> 512 idxs
        scatw = np.zeros((NS, N_HRAW, K_SC, 128), np.int16)
        for B in range(NS):
            for r in range(N_HRAW):
                for k in range(K_SC):
                    scatw[B, r, k, :] = scat[B * SB + r + N_HRAW * k]
        scatw = scatw.reshape(-1, K_SC * 128)
        scatw = np.ascontiguousarray(
            scatw.reshape(-1, K_SC * 8, 16).transpose(0, 2, 1)
        ).reshape(-1, 16, K_SC * 8)
        scatw = scatw.transpose(1, 0, 2).reshape(16, -1)
        scatw = np.tile(scatw, (8, 1))
        # mdw: [128 slot, NG_pad, KC, NMD, 2] bf16 (pair-duplicated)
        mdw = np.repeat(
            md.transpose(2, 0, 1, 3)[:, :, :, :, None], 2, axis=4)
        mdw = mdw.reshape(128, -1).astype(BF)
        in_maps.append(dict(gathw=gathw.astype(np.int16),
                            scatw=scatw.astype(np.int16),
                            mdw=mdw))

    W1s = reparam_weights(W1)
    W2s = reparam_weights(W2)
    w1dev = pack_wstack(W1s).astype(BF)
    w2dev = pack_wstack(W2s).astype(BF)
    bias1 = np.broadcast_to(np.asarray(b1, np.float32), (128, C)).copy()
    bias2 = np.broadcast_to(np.asarray(b2, np.float32), (128, C)).copy()
    root1d = np.asarray(root1).astype(BF)
    root2d = np.asarray(root2).astype(BF)

    for c in range(N_CORES):
        lo = c * NPC
        xo = np.zeros((NPC, C), np.float32)
        n_real = max(0, min(n_nodes - lo, NPC))
        if n_real > 0:
            xo[:n_real] = np.asarray(x, np.float32)[lo:lo + n_real]
        in_maps[c].update(dict(
            xown=xo.astype(BF),
            W1dev=w1dev, W2dev=w2dev,
            root1dev=root1d, root2dev=root2d,
            bias1rep=bias1, bias2rep=bias2,
        ))
    meta = dict(NG=NG_pad, NT=NT, n_src_rows=n_src_rows, NPC=NPC)
    return in_maps, meta


def build_program(NG_pad, NT, n_src_rows):
    NPC = NT * 128
    NB = NG_pad // GB
    NS = NG_pad // SB
    n_pairs = n_src_rows // 2
    PAIR_SEG = n_pairs // 2
    nc = bacc.Bacc("TRN2", target_bir_lowering=False, debug=False,
                   num_devices=N_CORES, num_swdge_queues=4)

    t_xown = nc.dram_tensor("xown", [NPC, C], BF16, kind="ExternalInput").ap()
    t_gathw = nc.dram_tensor("gathw", [128, NB * N_SUB * (SUB_SLOTS * GB // 16)],
                             I16, kind="ExternalInput").ap()
    t_scatw = nc.dram_tensor("scatw", [128, NS * N_HRAW * K_SC * 8], I16,
                             kind="ExternalInput").ap()
    t_mdw = nc.dram_tensor("mdw", [128, NG_pad * KC * NMD * 2], BF16,
                           kind="ExternalInput").ap()
    t_W1 = nc.dram_tensor("W1dev", [128, N_WCHUNK * C], BF16,
                          kind="ExternalInput").ap()
    t_W2 = nc.dram_tensor("W2dev", [128, N_WCHUNK * C], BF16,
                          kind="ExternalInput").ap()
    t_root1 = nc.dram_tensor("root1dev", [C, C], BF16, kind="ExternalInput").ap()
    t_root2 = nc.dram_tensor("root2dev", [C, C], BF16, kind="ExternalInput").ap()
    t_bias1 = nc.dram_tensor("bias1rep", [128, C], FP32, kind="ExternalInput").ap()
    t_bias2 = nc.dram_tensor("bias2rep", [128, C], FP32, kind="ExternalInput").ap()
    t_out = nc.dram_tensor("out", [NPC, C], FP32, kind="ExternalOutput").ap()

    t_xsrc_bf = nc.dram_tensor("xsrcbf", [n_src_rows, C], BF16,
                               kind="Internal", addr_space="Shared").ap()
    t_hfull_bf = nc.dram_tensor("hfullbf", [n_src_rows, C], BF16,
                                kind="Internal", addr_space="Shared").ap()

    with tile.TileContext(nc) as tc:
        import contextlib
        with contextlib.ExitStack() as ctx:
            dram = ctx.enter_context(tc.tile_pool(name="dram", bufs=1, space="DRAM"))
            const = ctx.enter_context(tc.tile_pool(name="const", bufs=1))
            pf = ctx.enter_context(tc.tile_pool(name="pf", bufs=2))
            data = ctx.enter_context(tc.tile_pool(name="data", bufs=2))
            oacc = ctx.enter_context(tc.tile_pool(name="oacc", bufs=2))
            sfin = ctx.enter_context(tc.tile_pool(name="sfin", bufs=3))
            pb = ctx.enter_context(tc.tile_pool(name="pb", bufs=3))
            ps_s = ctx.enter_context(tc.tile_pool(name="pss", bufs=1, space="PSUM"))
            ps_t = ctx.enter_context(tc.tile_pool(name="pst", bufs=2, space="PSUM"))
            ps_o = ctx.enter_context(tc.tile_pool(name="pso", bufs=2, space="PSUM"))

            h_raw_l = [[dram.tile([NPC + 128, C], FP32, name=f"hraw{li}_{i}")
                        for i in range(N_HRAW)] for li in range(2)]
            h_own_bf = dram.tile([NPC, C], BF16)
            xown_int = dram.tile([NPC, C], BF16)

            nc.sync.dma_start(xown_int[:, :], t_xown[:, :])
            if not _os.environ.get('K2_NOCOLL'):
                nc.gpsimd.collective_compute(
                    "AllGather", ALU.bypass,
                    replica_groups=[list(range(N_CORES))],
                    ins=[xown_int.opt()], outs=[t_xsrc_bf])

            iota_i = const.tile([128, 128], mybir.dt.int32)
            nc.gpsimd.iota(iota_i[:], pattern=[[1, 128]], base=0,
                           channel_multiplier=0)
            iota_b = const.tile([128, 128], BF16)
            nc.vector.tensor_copy(iota_b[:], iota_i[:])
            iota_pair = iota_b.rearrange("p (r t) -> p r t", t=2)
            ident = const.tile([128, 128], BF16)
            make_identity(nc, ident[:])
            zero_t = const.tile([128, 8, C], FP32)
            nc.vector.memset(zero_t[:], 0.0)
            w1_t = const.tile([128, N_WCHUNK, C], BF16)
            nc.sync.dma_start(w1_t[:], t_W1.rearrange("p (w c) -> p w c", c=C))
            w2_t = const.tile([128, N_WCHUNK, C], BF16)
            nc.sync.dma_start(w2_t[:], t_W2.rearrange("p (w c) -> p w c", c=C))
            root1_t = const.tile([C, C], BF16)
            nc.sync.dma_start(root1_t[:], t_root1[:])
            root2_t = const.tile([C, C], BF16)
            nc.sync.dma_start(root2_t[:], t_root2[:])
            bias1_sm = const.tile([128, C], FP32)
            nc.sync.dma_start(bias1_sm[:], t_bias1[:])
            bias2_sm = const.tile([128, C], FP32)
            nc.sync.dma_start(bias2_sm[:], t_bias2[:])
            bias1_t = const.tile([128, 8, C], FP32)
            nc.scalar.activation(
                bias1_t[:], bias1_sm[:, None, :].to_broadcast([128, 8, C]),
                AF.Copy)
            bias2_t = const.tile([128, 8, C], FP32)
            nc.scalar.activation(
                bias2_t[:], bias2_sm[:, None, :].to_broadcast([128, 8, C]),
                AF.Copy)
            scat_all = const.tile([128, NS * N_HRAW * K_SC * 8], I16)
            nc.sync.dma_start(scat_all[:], t_scatw[:])

            mdw_r = t_mdw.rearrange("p (g k m t) -> p g k m t",
                                    k=KC, m=NMD, t=2)
            GCOL = SUB_SLOTS * GB // 16
            gath_r = t_gathw.rearrange("p (b c) -> p b c", c=GCOL)

            def layer(src_table, w_t, root_t, bias_t, out_dram, out_dtype,
                      h_raw, xo_src):
                pair_view = src_table.rearrange("(r t) c -> r (t c)", t=2)
                # fill accumulators: bias pre-folded into h_raw[0], rest zero
                NTF = NT + 1
                for hi, hb in enumerate(h_raw):
                    fill = bias_t if hi == 0 else zero_t
                    t = 0
                    while t < NTF:
                        n = min(8, NTF - t)
                        dst = hb[t * 128:(t + n) * 128, :].rearrange(
                            "(t p) c -> p t c", p=128)
                        nc.sync.dma_start(dst, fill[:, 0:n, :])
                        t += n

                for b in range(NB):
                    # prefetch tables for this batch
                    md_pf = pf.tile([128, GB, KC, NMD, 2], BF16, tag="md")
                    nc.sync.dma_start(
                        md_pf[:], mdw_r[:, b * GB:(b + 1) * GB, :, :, :])
                    gath_pf = pf.tile([128, N_SUB, GCOL], I16, tag="gw")
                    nc.sync.dma_start(
                        gath_pf[:], gath_r[:, b * N_SUB:(b + 1) * N_SUB, :])
                    # batched gathers: queue q covers sub-list q of 4 groups
                    xs4 = data.tile([128, N_SUB, KC // N_SUB * GB, 128], BF16,
                                    tag="xs4")
                    for q in range(N_SUB):
                        s = q % 2
                        nc.gpsimd.dma_gather(
                            out_ap=xs4[:, q, :, :],
                            in_ap=pair_view[s * PAIR_SEG:(s + 1) * PAIR_SEG, :],
                            idxs_ap=gath_pf[:, q, :],
                            num_idxs=SUB_SLOTS * GB,
                            num_idxs_reg=SUB_SLOTS * GB,
                            elem_size=128,
                            queue_num=q)

                    for j in range(GB):
                        g = b * GB + j
                        md_g = md_pf[:, j, :, :, :]
                        # one-hots: oh, oh*v, oh*v2  [128, KC, 3, 128]
                        oh3 = data.tile([128, KC, 3, 128], BF16, tag="oh3")
                        oh3p = oh3.rearrange("p k b (r t) -> p k b r t", t=2)
                        nc.vector.tensor_tensor(
                            out=oh3p[:, :, 0, :, :],
                            in0=iota_pair[:, None, :, :].to_broadcast(
                                [128, KC, 64, 2]),
                            in1=md_g[:, :, 0, None, :].to_broadcast(
                                [128, KC, 64, 2]),
                            op=ALU.is_equal)
                        nc.vector.tensor_tensor(
                            out=oh3p[:, :, 1, :, :],
                            in0=oh3p[:, :, 0, :, :],
                            in1=md_g[:, :, 3, None, :].to_broadcast(
                                [128, KC, 64, 2]),
                            op=ALU.mult)
                        nc.vector.tensor_tensor(
                            out=oh3p[:, :, 2, :, :],
                            in0=oh3p[:, :, 0, :, :],
                            in1=md_g[:, :, 4, None, :].to_broadcast(
                                [128, KC, 64, 2]),
                            op=ALU.mult)
                        # z = [x, x*u, x*u2]; z0 on scalar, z1/z2 on DVE
                        zst = data.tile([128, KC, 3, 64], BF16, tag="zst")
                        zstp = zst.rearrange("p k a (r t) -> p k a r t", t=2)
                        for half in range(2):
                            ks = slice(half * (KC // 2), (half + 1) * (KC // 2))
                            h0 = half * 64
                            xsv = xs4[:, half * 2:(half + 1) * 2, :, :]
                            xsv = xsv.rearrange(
                                "p q (c j) e -> p q c j e", j=GB)[:, :, :, j,
                                                                  h0:h0 + 64]
                            nc.scalar.activation(
                                zst[:, ks, 0, :],
                                xsv.rearrange("p q c e -> p (q c) e"), AF.Copy)
                            xsp = xsv.rearrange(
                                "p q c (r t) -> p (q c) r t", t=2)
                            for ai, mi in ((1, 1), (2, 2)):
                                nc.vector.tensor_tensor(
                                    out=zstp[:, ks, ai, :, :],
                                    in0=xsp[:],
                                    in1=md_g[:, ks, mi, None, :].to_broadcast(
                                        [128, KC // 2, 32, 2]),
                                    op=ALU.mult)

                        s_ps = [ps_s.tile([128, 3 * C], FP32, tag=f"s{bb}",
                                          name=f"sps{bb}")
                                for bb in range(3)]
                        for t in range(KC):
                            z_t = zst[:, t, :, :].rearrange("p a c -> p (a c)")
                            for bb in range(3):
                                nc.tensor.matmul(
                                    s_ps[bb][:], oh3[:, t, bb, :], z_t,
                                    start=(t == 0), stop=(t == KC - 1))

                        s_sb = sfin.tile([128, 640], BF16, tag="ssb")
                        for bb in range(3):
                            nc.scalar.activation(
                                s_sb[:, bb * 192:(bb + 1) * 192],
                                s_ps[bb][:], AF.Copy)
                        nc.vector.memset(s_sb[:, 576:640], 0.0)

                        o_ps = ps_o.tile([128, C], FP32, tag="ops")
                        sst_sb = sfin.tile([128, N_WCHUNK, 128], BF16,
                                           tag="sst")
                        for jj in range(N_WCHUNK):
                            st_ps = ps_t.tile([128, 128], BF16, tag="stps")
                            nc.tensor.transpose(
                                st_ps[:], s_sb[:, jj * 128:(jj + 1) * 128],
                                ident[:])
                            nc.scalar.activation(sst_sb[:, jj, :], st_ps[:],
                                                 AF.Copy)
                        for jj in range(N_WCHUNK):
                            nc.tensor.matmul(o_ps[:], sst_sb[:, jj, :],
                                             w_t[:, jj, :],
                                             start=(jj == 0),
                                             stop=(jj == N_WCHUNK - 1))
                        r = g % N_HRAW
                        kpos = (g % SB) // N_HRAW
                        if g % SB == 0:
                            layer.o_tiles = [
                                oacc.tile([128, K_SC, C], FP32,
                                          tag=f"oacc{rr}", name=f"ot{rr}")
                                for rr in range(N_HRAW)]
                        nc.scalar.activation(layer.o_tiles[r][:, kpos, :],
                                             o_ps[:], AF.Copy)
                        if g % SB == SB - 1:
                            B = g // SB
                            for rr in range(N_HRAW):
                                nc.gpsimd.dma_scatter_add(
                                    out_ap=h_raw[rr][:],
                                    in_ap=layer.o_tiles[rr][:],
                                    idxs_ap=scat_all[:, (B * N_HRAW + rr) * K_SC * 8:
                                                     (B * N_HRAW + rr + 1) * K_SC * 8],
                                    num_idxs=K_SC * 128,
                                    num_idxs_reg=K_SC * 128,
                                    elem_size=C,
                                    queue_num=rr)

                # ---- pass B ----
                TB = 7
                assert NT % TB == 0
                for blk in range(NT // TB):
                    rows = slice(blk * TB * 128, (blk + 1) * TB * 128)

                    def bview(buf):
                        return buf[rows, :].rearrange("(t p) c -> p t c",
                                                      p=128)

                    ha = pb.tile([128, TB, C], FP32, tag="ha")
                    nc.sync.dma_start(ha[:], bview(h_raw[0]))
                    hb2 = pb.tile([128, TB, C], FP32, tag="hb2")
                    nc.sync.dma_start(hb2[:], bview(h_raw[1]))
                    xo_t = pb.tile([128, TB, C], BF16, tag="xo")
                    nc.sync.dma_start(xo_t[:], bview(xo_src))
                    r_ps = ps_s.tile([128, TB, C], FP32, tag="rps")
                    for jj in range(TB):
                        xoT_ps = ps_t.tile([64, 128], BF16, tag="stps")
                        nc.tensor.transpose(xoT_ps[:], xo_t[:, jj, :],
                                            ident[:])
                        xoT_sb = pb.tile([64, 128], BF16, tag="xoTsb")
                        nc.scalar.activation(xoT_sb[:], xoT_ps[:], AF.Copy)
                        nc.tensor.matmul(r_ps[:, jj, :], xoT_sb[:], root_t[:],
                                         start=True, stop=True)
                    s1 = pb.tile([128, TB, C], FP32, tag="s1")
                    nc.gpsimd.tensor_tensor(out=s1[:], in0=ha[:], in1=hb2[:],
                                            op=ALU.add)
                    s4 = pb.tile([128, TB, C], FP32, tag="s4")
                    nc.vector.tensor_tensor(out=s4[:], in0=s1[:], in1=r_ps[:],
                                            op=ALU.add)
                    h_t = pb.tile([128, TB, C], out_dtype, tag="ht")
                    nc.scalar.activation(h_t[:], s4[:], AF.Relu)
                    nc.sync.dma_start(bview(out_dram), h_t[:])

            if _os.environ.get('K2_L1'):
                layer(t_xsrc_bf, w1_t, root1_t, bias1_t, t_out, FP32,
                      h_raw_l[0], xown_int)
            else:
                layer(t_xsrc_bf, w1_t, root1_t, bias1_t, h_own_bf, BF16,
                      h_raw_l[0], xown_int)
                if not _os.environ.get('K2_NOCOLL'):
                    nc.gpsimd.collective_compute(
                        "AllGather", ALU.bypass,
                        replica_groups=[list(range(N_CORES))],
                        ins=[h_own_bf.opt()], outs=[t_hfull_bf])
                layer(t_hfull_bf, w2_t, root2_t, bias2_t, t_out, FP32,
                      h_raw_l[1], h_own_bf)

    nc.compile()
    return nc


_CACHE = {}


def _get_program(NG_pad, NT_, n_src_rows):
    key = (NG_pad, NT_, n_src_rows)
    if key not in _CACHE:
        _CACHE[key] = build_program(NG_pad, NT_, n_src_rows)
    return _CACHE[key]


# ---------------------------------------------------------------------------
# Cached PJRT runner: jitted executable built once; static inputs stay
# device-resident; donated output buffers ping-pong from the previous call.
# ---------------------------------------------------------------------------

_DYNAMIC_INPUTS = ("xown",)


class _Runner:
    def __init__(self, nc, n_cores, dynamic_names=_DYNAMIC_INPUTS):
        import jax
        from jax.experimental.shard_map import shard_map
        from jax.sharding import Mesh, PartitionSpec, NamedSharding
        from concourse import bass2jax

        bass2jax.install_neuronx_cc_hook()
        self.jax = jax
        self.nc = nc
        self.n_cores = n_cores
        partition_name = (nc.partition_id_tensor.name
                          if nc.partition_id_tensor else None)
        in_names, out_names, out_avals, zero_outs = [], [], [], []
        for alloc in nc.m.functions[0].allocations:
            if not isinstance(alloc, mybir.MemoryLocationSet):
                continue
            name = alloc.memorylocations[0].name
            if alloc.kind == "ExternalInput":
                if name != partition_name:
                    in_names.append(name)
            elif alloc.kind == "ExternalOutput":
                shape = tuple(alloc.tensor_shape)
                dtype = mybir.dt.np(alloc.dtype)
                out_avals.append(jax.core.ShapedArray(shape, dtype))
                out_names.append(name)
                zero_outs.append(np.zeros(shape, dtype))
        assert nc.dbg_addr is None or not nc.dbg_callbacks
        self.dynamic_names = tuple(n for n in dynamic_names if n in in_names)
        self.in_names = in_names
        self.out_names = out_names
        self.out_avals = out_avals
        self.zero_outs = zero_outs
        n_params = len(in_names)
        n_outs = len(out_names)
        in_names_all = in_names + out_names
        if partition_name is not None:
            in_names_all.append(partition_name)
        donate = tuple(range(n_params, n_params + n_outs))

        def _body(*args):
            operands = list(args)
            if partition_name is not None:
                operands.append(bass2jax.partition_id_tensor())
            outs = bass2jax._bass_exec_p.bind(
                *operands,
                out_avals=tuple(out_avals),
                in_names=tuple(in_names_all),
                out_names=tuple(out_names),
                lowering_input_output_aliases=(),
                sim_require_finite=True,
                sim_require_nnan=True,
                nc=nc,
            )
            return tuple(outs)

        devices = jax.devices()[:n_cores]
        assert len(devices) == n_cores
        mesh = Mesh(np.asarray(devices), ("core",))
        self.sharding = NamedSharding(mesh, PartitionSpec("core"))
        in_specs = (PartitionSpec("core"),) * (n_params + n_outs)
        out_specs = (PartitionSpec("core"),) * n_outs
        self.fn = jax.jit(
            shard_map(_body, mesh=mesh, in_specs=in_specs,
                      out_specs=out_specs, check_rep=False),
            donate_argnums=donate, keep_unused=True)
        self.static_arrs = {}   # name -> device array
        self.prev_outs = None   # device arrays to donate as output buffers

    def _concat(self, in_maps, name):
        return np.concatenate(
            [np.asarray(in_maps[c][name]) for c in range(self.n_cores)], axis=0)

    def stage_static(self, in_maps):
        for name in self.in_names:
            if name in self.dynamic_names or name in self.static_arrs:
                continue
            darr = self.jax.device_put(self._concat(in_maps, name),
                                       self.sharding)
            darr.block_until_ready()
            self.static_arrs[name] = darr

    def run(self, in_maps):
        jax = self.jax
        t0 = _time.time()
        dyn = {}
        for name in self.dynamic_names:
            darr = jax.device_put(self._concat(in_maps, name), self.sharding)
            dyn[name] = darr
        for d in dyn.values():
            d.block_until_ready()
        t1 = _time.time()
        args = [dyn[n] if n in self.dynamic_names else self.static_arrs[n]
                for n in self.in_names]
        if self.prev_outs is None:
            outs_bufs = [
                jax.device_put(
                    np.zeros((self.n_cores * z.shape[0], *z.shape[1:]), z.dtype),
                    self.sharding)
                for z in self.zero_outs]
        else:
            outs_bufs = self.prev_outs
        t2 = _time.time()
        outs = self.fn(*args, *outs_bufs)
        jax.block_until_ready(outs)
        t3 = _time.time()
        self.prev_outs = list(outs)
        results = [np.asarray(o) for o in outs]
        t4 = _time.time()
        self.stage_seconds = t1 - t0
        self.exec_seconds = t3 - t2
        self.fetch_seconds = t4 - t3
        self.total_seconds = t4 - t0
        return {name: results[i].reshape(self.n_cores, *self.out_avals[i].shape)
                for i, name in enumerate(self.out_names)}

    def time_marginal_exec(self, in_maps, iters=10):
        """Per-execution device time via pipelined launches: executions are
        chained through the donated output buffer, so they serialize on
        device; (T_iters - T_1)/(iters-1) cancels the host-tunnel sync RTT.
        Returns (seconds_per_exec, results_dict) with results from the last
        execution (identical inputs -> identical output each run)."""
        jax = self.jax
        assert self.prev_outs is not None, "call run() first to warm"
        dyn = {name: jax.device_put(self._concat(in_maps, name), self.sharding)
               for name in self.dynamic_names}
        for d in dyn.values():
            d.block_until_ready()
        args = [dyn[n] if n in self.dynamic_names else self.static_arrs[n]
                for n in self.in_names]
        outs = self.prev_outs

        def timed_chain(n, outs):
            t0 = _time.time()
            for _ in range(n):
                outs = self.fn(*args, *outs)
            jax.block_until_ready(outs)
            return _time.time() - t0, outs

        # two chain lengths; slope cancels the tunnel sync RTT
        n_lo, n_hi = iters, iters * 5
        t_lo, outs = timed_chain(n_lo, outs)
        t_hi, outs = timed_chain(n_hi, outs)
        self.prev_outs = list(outs)
        per_exec = max(0.0, (t_hi - t_lo) / (n_hi - n_lo))
        self.marginal_detail = (n_lo, t_lo, n_hi, t_hi)
        results = [np.asarray(o) for o in outs]
        res = {name: results[i].reshape(self.n_cores, *self.out_avals[i].shape)
               for i, name in enumerate(self.out_names)}
        return per_exec, res



_RUNNERS = {}


def _get_runner(nc, key):
    if key not in _RUNNERS:
        _RUNNERS[key] = _Runner(nc, N_CORES)
    return _RUNNERS[key]


def _assemble_out(res, NPC):
    out_pc = res['out']
    out = np.zeros((N_NODES, C), np.float32)
    for c in range(N_CORES):
        lo = c * NPC
        n_real = max(0, min(N_NODES - lo, NPC))
        if n_real > 0:
            out[lo:lo + n_real] = out_pc[c][:n_real]
    return out


def measure_exec(x, edge_index, pseudo, W1, root1, b1, W2, root2, b2,
                 iters=10):
    in_maps, meta = prep(edge_index, pseudo, x, W1, root1, b1,
                         W2, root2, b2, N_NODES, NT)
    key = (meta['NG'], meta['NT'], meta['n_src_rows'])
    nc = _get_program(*key)
    runner = _get_runner(nc, key)
    runner.stage_static(in_maps)
    if runner.prev_outs is None:
        runner.run(in_maps)
    per_exec, res = runner.time_marginal_exec(in_maps, iters=iters)
    return per_exec, _assemble_out(res, meta['NPC'])


def kernel(x, edge_index, pseudo, W1, root1, b1, W2, root2, b2):
    in_maps, meta = prep(edge_index, pseudo, x, W1, root1, b1,
                         W2, root2, b2, N_NODES, NT)
    key = (meta['NG'], meta['NT'], meta['n_src_rows'])
    nc = _get_program(*key)
    runner = _get_runner(nc, key)
    runner.stage_static(in_maps)
    _t0 = _time.time()
    res = runner.run(in_maps)
    kernel.last_total_seconds = _time.time() - _t0
    kernel.last_exec_seconds = runner.exec_seconds
    kernel.last_stage_seconds = runner.stage_seconds
    kernel.last_fetch_seconds = runner.fetch_seconds
    return _assemble_out(res, meta['NPC'])


# revision 4
# speedup vs baseline: 1.2679x; 1.2679x over previous
"""Trainium2 Bass kernel v3 for nn_MeshDownConv (2-layer SplineConv GNN).

Same edge-parallel dst-window design as kernel.py, plus:
- bf16 pair-row gathers (table [50176,128] bf16, 256B rows; parity picks the
  64-col half) -> no f32 expand pass, no per-group cast.
- dma_gather batched over 4 groups per instruction (SWDGE emission is ~1us
  fixed per instr).
- dma_scatter_add batched over 4 same-residue groups (disjoint rows).
- za/ohb factorization: DVE builds [x*u, x*u2] plus 3 weighted one-hots
  (oh, oh*v, oh*v2); 6 matmuls/chunk accumulate S[b][node, (a,c)].
- per-batch prefetch of gather-idx and (dstl,u,u2,v,v2) tables from DRAM.
"""
import sys
sys.path.insert(0, '/opt/trn_rl_repo')
import time as _time
import numpy as np
import ml_dtypes

import concourse.bass as bass
import concourse.bacc as bacc
import concourse.mybir as mybir
import concourse.tile as tile
from concourse.masks import make_identity

N_NODES = 100000
N_EDGES = 1600000
NT = 98

BF = ml_dtypes.bfloat16
FP32 = mybir.dt.float32
BF16 = mybir.dt.bfloat16
I16 = mybir.dt.int16
AF = mybir.ActivationFunctionType
ALU = mybir.AluOpType

C = 64
NUM_Q = 9
QDIM = 576                  # 9 * 64 monomial features
N_WCHUNK = 5
N_CORES = 8
WINDOW = 128
N_SUB = 4                   # sub-lists: (parity, seg-half)
KC = 16                     # chunks (128-slot columns) per group
SUB_COLS = KC // N_SUB      # 4 columns per sub-list
SUB_SLOTS = SUB_COLS * 128  # 512 slots per sub-list per group
N_HRAW = 2
import os as _os
GB = int(_os.environ.get('K2_GB', '2'))   # groups per gather batch
SB = int(_os.environ.get('K2_SB', '8'))   # groups per scatter block
K_SC = SB // N_HRAW         # groups per scatter instruction
NMD = 5                     # per-slot table: dstl, u, u2, v, v2

P_BASIS = np.array([[0.5, -1.0, 0.5],
                    [0.5, 1.0, -1.0],
                    [0.0, 0.0, 0.5]], dtype=np.float64)


def reparam_weights(W):
    # monomial-basis W, rows ordered (b, a, c): row = (b*3+a)*C + c
    W33 = np.asarray(W, np.float64).reshape(3, 3, C, C)
    Wp = np.einsum('ia,jb,ijcd->abcd', P_BASIS, P_BASIS, W33)  # [a,b,cin,cout]
    Wp = Wp.transpose(1, 0, 2, 3)                              # [b,a,cin,cout]
    return Wp.reshape(QDIM, C)


def pack_wstack(Wstk):
    # 5 chunks of 128 rows; chunk4 = rows 512:576 + 64 zero rows
    out = np.zeros((128, N_WCHUNK, C), np.float32)
    for j in range(N_WCHUNK):
        w0 = j * 128
        n = min(128, QDIM - w0)
        out[:n, j, :] = Wstk[w0:w0 + n]
    return out.reshape(128, N_WCHUNK * C)


def pack_groups_core(sub, dl, NPC):
    """Greedy-pack dst-sorted edges into <=128-node windows with per-sublist
    512-slot budgets. sub = sub-list key per edge, dl = local dst."""
    node_starts = np.searchsorted(dl, np.arange(NPC + 1))
    groups = []
    n = 0
    while n < NPC:
        if node_starts[n] == node_starts[NPC]:
            break
        w = n
        cnt = np.zeros(N_SUB, np.int64)
        while n < NPC and n - w < WINDOW:
            ea, eb = node_starts[n], node_starts[n + 1]
            if ea == eb:
                n += 1
                continue
            add = np.bincount(sub[ea:eb], minlength=N_SUB)
            if np.any(cnt + add > SUB_SLOTS):
                break
            cnt += add
            n += 1
        assert cnt.sum() > 0, "single node exceeds sub-list budget"
        ea, eb = node_starts[w], node_starts[n]
        idxs = np.arange(ea, eb)
        per_sub = [idxs[sub[ea:eb] == q] for q in range(N_SUB)]
        groups.append((w, per_sub))
    return groups


def prep(edge_index, pseudo, x, W1, root1, b1, W2, root2, b2, n_nodes, NT):
    NPC = NT * 128
    n_src_rows = N_CORES * NPC
    n_pairs = n_src_rows // 2
    PAIR_SEG = n_pairs // 2  # 25088

    src = np.asarray(edge_index[0], np.int64)
    dst = np.asarray(edge_index[1], np.int64)
    order = np.argsort(dst, kind='stable')
    src = src[order]
    dst = dst[order]
    u = np.asarray(pseudo)[order, 0].astype(np.float64)
    v = np.asarray(pseudo)[order, 1].astype(np.float64)

    # sub-list key: parity*2 + seg-half (parity -> which 64-col half)
    par = (src % 2).astype(np.int64)
    seg = ((src // 2) // PAIR_SEG).astype(np.int64)
    sub_all = par * 2 + seg
    pair_idx = (src // 2) - seg * PAIR_SEG  # int16-safe

    core_groups = []
    for c in range(N_CORES):
        lo, hi = c * NPC, min(n_nodes, (c + 1) * NPC)
        a, b = np.searchsorted(dst, lo), np.searchsorted(dst, hi)
        g = pack_groups_core(sub_all[a:b], dst[a:b] - lo, NPC)
        core_groups.append((a, g))
    NG = max(len(g) for _, g in core_groups)
    NG_pad = ((NG + SB - 1) // SB) * SB
    NB = NG_pad // GB
    NS = NG_pad // SB

    in_maps = []
    for c in range(N_CORES):
        a, groups = core_groups[c]
        lo = c * NPC
        # per (group, sublist): 512 idxs (pad with 0) + per-slot md values
        gidx = np.zeros((NG_pad, N_SUB, SUB_SLOTS), np.int16)
        md = np.zeros((NG_pad, KC, 128, NMD), np.float64)
        md[:, :, :, 0] = -1.0  # pad dstl never matches iota
        scat = np.zeros((NG_pad, 128), np.int64)
        wins = np.zeros(NG_pad, np.int64)
        for g in range(NG_pad):
            if g < len(groups):
                w, per_sub = groups[g]
                wins[g] = w
                scat[g] = w + np.arange(128)
                for q in range(N_SUB):
                    es = per_sub[q]  # edge idx into core list
                    k = len(es)
                    gidx[g, q, :k] = pair_idx[a + es]
                    i = np.arange(k)
                    t = q * SUB_COLS + i // 128
                    s = i % 128
                    md[g, t, s, 0] = (dst[a + es] - lo - w).astype(np.float64)
                    md[g, t, s, 1] = u[a + es]
                    md[g, t, s, 2] = u[a + es] ** 2
                    md[g, t, s, 3] = v[a + es]
                    md[g, t, s, 4] = v[a + es] ** 2
            else:
                scat[g] = NPC + np.arange(128)  # junk rows
        # gathw: per (batch, q): 2048 idxs ordered (colq, grp within batch)
        gathw = np.zeros((NB, N_SUB, SUB_COLS, GB, 128), np.int16)
        for b in range(NB):
            for q in range(N_SUB):
                for j in range(GB):
                    gathw[b, q, :, j, :] = gidx[b * GB + j, q].reshape(
                        SUB_COLS, 128)
        gcol = SUB_SLOTS * GB // 16
        gathw = gathw.reshape(NB * N_SUB, SUB_SLOTS * GB)
        gathw = np.ascontiguousarray(
            gathw.reshape(-1, gcol, 16).transpose(0, 2, 1)
        ).reshape(-1, 16, gcol)
        gathw = gathw.transpose(1, 0, 2).reshape(16, -1)
        gathw = np.tile(gathw, (8, 1))
        # scatw: per (block, residue): 4 groups (r, r+3, r+6, r+9) -> 512 idxs
        scatw = np.zeros((NS, N_HRAW, K_SC, 128), np.int16)
        for B in range(NS):
            for r in range(N_HRAW):
                for k in range(K_SC):
                    scatw[B, r, k, :] = scat[B * SB + r + N_HRAW * k]
        scatw = scatw.reshape(-1, K_SC * 128)
        scatw = np.ascontiguousarray(
            scatw.reshape(-1, K_SC * 8, 16).transpose(0, 2, 1)
        ).reshape(-1, 16, K_SC * 8)
        scatw = scatw.transpose(1, 0, 2).reshape(16, -1)
        scatw = np.tile(scatw, (8, 1))
        # mdw: [128 slot, NG_pad, KC, NMD, 2] bf16 (pair-duplicated)
        mdw = np.repeat(
            md.transpose(2, 0, 1, 3)[:, :, :, :, None], 2, axis=4)
        mdw = mdw.reshape(128, -1).astype(BF)
        in_maps.append(dict(gathw=gathw.astype(np.int16),
                            scatw=scatw.astype(np.int16),
                            mdw=mdw))

    W1s = reparam_weights(W1)
    W2s = reparam_weights(W2)
    w1dev = pack_wstack(W1s).astype(BF)
    w2dev = pack_wstack(W2s).astype(BF)
    bias1 = np.broadcast_to(np.asarray(b1, np.float32), (128, C)).copy()
    bias2 = np.broadcast_to(np.asarray(b2, np.float32), (128, C)).copy()
    root1d = np.asarray(root1).astype(BF)
    root2d = np.asarray(root2).astype(BF)

    for c in range(N_CORES):
        lo = c * NPC
        xo = np.zeros((NPC, C), np.float32)
        n_real = max(0, min(n_nodes - lo, NPC))
        if n_real > 0:
            xo[:n_real] = np.asarray(x, np.float32)[lo:lo + n_real]
        in_maps[c].update(dict(
            xown=xo.astype(BF),
            W1dev=w1dev, W2dev=w2dev,
            root1dev=root1d, root2dev=root2d,
            bias1rep=bias1, bias2rep=bias2,
        ))
    meta = dict(NG=NG_pad, NT=NT, n_src_rows=n_src_rows, NPC=NPC)
    return in_maps, meta


def build_program(NG_pad, NT, n_src_rows):
    NPC = NT * 128
    NB = NG_pad // GB
    NS = NG_pad // SB
    n_pairs = n_src_rows // 2
    PAIR_SEG = n_pairs // 2
    nc = bacc.Bacc("TRN2", target_bir_lowering=False, debug=False,
                   num_devices=N_CORES, num_swdge_queues=4)

    t_xown = nc.dram_tensor("xown", [NPC, C], BF16, kind="ExternalInput").ap()
    t_gathw = nc.dram_tensor("gathw", [128, NB * N_SUB * (SUB_SLOTS * GB // 16)],
                             I16, kind="ExternalInput").ap()
    t_scatw = nc.dram_tensor("scatw", [128, NS * N_HRAW * K_SC * 8], I16,
                             kind="ExternalInput").ap()
    t_mdw = nc.dram_tensor("mdw", [128, NG_pad * KC * NMD * 2], BF16,
                           kind="ExternalInput").ap()
    t_W1 = nc.dram_tensor("W1dev", [128, N_WCHUNK * C], BF16,
                          kind="ExternalInput").ap()
    t_W2 = nc.dram_tensor("W2dev", [128, N_WCHUNK * C], BF16,
                          kind="ExternalInput").ap()
    t_root1 = nc.dram_tensor("root1dev", [C, C], BF16, kind="ExternalInput").ap()
    t_root2 = nc.dram_tensor("root2dev", [C, C], BF16, kind="ExternalInput").ap()
    t_bias1 = nc.dram_tensor("bias1rep", [128, C], FP32, kind="ExternalInput").ap()
    t_bias2 = nc.dram_tensor("bias2rep", [128, C], FP32, kind="ExternalInput").ap()
    t_out = nc.dram_tensor("out", [NPC, C], FP32, kind="ExternalOutput").ap()

    t_xsrc_bf = nc.dram_tensor("xsrcbf", [n_src_rows, C], BF16,
                               kind="Internal", addr_space="Shared").ap()
    t_hfull_bf = nc.dram_tensor("hfullbf", [n_src_rows, C], BF16,
                                kind="Internal", addr_space="Shared").ap()

    with tile.TileContext(nc) as tc:
        import contextlib
        with contextlib.ExitStack() as ctx:
            dram = ctx.enter_context(tc.tile_pool(name="dram", bufs=1, space="DRAM"))
            const = ctx.enter_context(tc.tile_pool(name="const", bufs=1))
            pf = ctx.enter_context(tc.tile_pool(name="pf", bufs=2))
            data = ctx.enter_context(tc.tile_pool(name="data", bufs=2))
            oacc = ctx.enter_context(tc.tile_pool(name="oacc", bufs=2))
            sfin = ctx.enter_context(tc.tile_pool(name="sfin", bufs=3))
            pb = ctx.enter_context(tc.tile_pool(name="pb", bufs=3))
            ps_s = ctx.enter_context(tc.tile_pool(name="pss", bufs=1, space="PSUM"))
            ps_t = ctx.enter_context(tc.tile_pool(name="pst", bufs=2, space="PSUM"))
            ps_o = ctx.enter_context(tc.tile_pool(name="pso", bufs=2, space="PSUM"))

            h_raw_l = [[dram.tile([NPC + 128, C], FP32, name=f"hraw{li}_{i}")
                        for i in range(N_HRAW)] for li in range(2)]
            h_own_bf = dram.tile([NPC, C], BF16)
            xown_int = dram.tile([NPC, C], BF16)

            nc.sync.dma_start(xown_int[:, :], t_xown[:, :])
            if not _os.environ.get('K2_NOCOLL'):
                nc.gpsimd.collective_compute(
                    "AllGather", ALU.bypass,
                    replica_groups=[list(range(N_CORES))],
                    ins=[xown_int.opt()], outs=[t_xsrc_bf])

            iota_i = const.tile([128, 128], mybir.dt.int32)
            nc.gpsimd.iota(iota_i[:], pattern=[[1, 128]], base=0,
                           channel_multiplier=0)
            iota_b = const.tile([128, 128], BF16)
            nc.vector.tensor_copy(iota_b[:], iota_i[:])
            iota_pair = iota_b.rearrange("p (r t) -> p r t", t=2)
            ident = const.tile([128, 128], BF16)
            make_identity(nc, ident[:])
            zero_t = const.tile([128, 8, C], FP32)
            nc.vector.memset(zero_t[:], 0.0)
            w1_t = const.tile([128, N_WCHUNK, C], BF16)
            nc.sync.dma_start(w1_t[:], t_W1.rearrange("p (w c) -> p w c", c=C))
            w2_t = const.tile([128, N_WCHUNK, C], BF16)
            nc.sync.dma_start(w2_t[:], t_W2.rearrange("p (w c) -> p w c", c=C))
            root1_t = const.tile([C, C], BF16)
            nc.sync.dma_start(root1_t[:], t_root1[:])
            root2_t = const.tile([C, C], BF16)
            nc.sync.dma_start(root2_t[:], t_root2[:])
            bias1_sm = const.tile([128, C], FP32)
            nc.sync.dma_start(bias1_sm[:], t_bias1[:])
            bias2_sm = const.tile([128, C], FP32)
            nc.sync.dma_start(bias2_sm[:], t_bias2[:])
            bias1_t = const.tile([128, 8, C], FP32)
            nc.scalar.activation(
                bias1_t[:], bias1_sm[:, None, :].to_broadcast([128, 8, C]),
                AF.Copy)
            bias2_t = const.tile([128, 8, C], FP32)
            nc.scalar.activation(
                bias2_t[:], bias2_sm[:, None, :].to_broadcast([128, 8, C]),
                AF.Copy)
            scat_all = const.tile([128, NS * N_HRAW * K_SC * 8], I16)
            nc.sync.dma_start(scat_all[:], t_scatw[:])

            mdw_r = t_mdw.rearrange("p (g k m t) -> p g k m t",
                                    k=KC, m=NMD, t=2)
            GCOL = SUB_SLOTS * GB // 16
            gath_r = t_gathw.rearrange("p (b c) -> p b c", c=GCOL)

            def layer(src_table, w_t, root_t, bias_t, out_dram, out_dtype,
                      h_raw, xo_src):
                pair_view = src_table.rearrange("(r t) c -> r (t c)", t=2)
                # fill accumulators: bias pre-folded into h_raw[0], rest zero
                NTF = NT + 1
                for hi, hb in enumerate(h_raw):
                    fill = bias_t if hi == 0 else zero_t
                    t = 0
                    while t < NTF:
                        n = min(8, NTF - t)
                        dst = hb[t * 128:(t + n) * 128, :].rearrange(
                            "(t p) c -> p t c", p=128)
                        nc.sync.dma_start(dst, fill[:, 0:n, :])
                        t += n

                for b in range(NB):
                    # prefetch tables for this batch
                    md_pf = pf.tile([128, GB, KC, NMD, 2], BF16, tag="md")
                    nc.sync.dma_start(
                        md_pf[:], mdw_r[:, b * GB:(b + 1) * GB, :, :, :])
                    gath_pf = pf.tile([128, N_SUB, GCOL], I16, tag="gw")
                    nc.sync.dma_start(
                        gath_pf[:], gath_r[:, b * N_SUB:(b + 1) * N_SUB, :])
                    # batched gathers: queue q covers sub-list q of 4 groups
                    xs4 = data.tile([128, N_SUB, KC // N_SUB * GB, 128], BF16,
                                    tag="xs4")
                    for q in range(N_SUB):
                        s = q % 2
                        nc.gpsimd.dma_gather(
                            out_ap=xs4[:, q, :, :],
                            in_ap=pair_view[s * PAIR_SEG:(s + 1) * PAIR_SEG, :],
                            idxs_ap=gath_pf[:, q, :],
                            num_idxs=SUB_SLOTS * GB,
                            num_idxs_reg=SUB_SLOTS * GB,
                            elem_size=128,
                            queue_num=q)

                    for j in range(GB):
                        g = b * GB + j
                        md_g = md_pf[:, j, :, :, :]
                        # one-hots: oh, oh*v, oh*v2  [128, KC, 3, 128]
                        oh3 = data.tile([128, KC, 3, 128], BF16, tag="oh3")
                        oh3p = oh3.rearrange("p k b (r t) -> p k b r t", t=2)
                        nc.vector.tensor_tensor(
                            out=oh3p[:, :, 0, :, :],
                            in0=iota_pair[:, None, :, :].to_broadcast(
                                [128, KC, 64, 2]),
                            in1=md_g[:, :, 0, None, :].to_broadcast(
                                [128, KC, 64, 2]),
                            op=ALU.is_equal)
                        nc.vector.tensor_tensor(
                            out=oh3p[:, :, 1, :, :],
                            in0=oh3p[:, :, 0, :, :],
                            in1=md_g[:, :, 3, None, :].to_broadcast(
                                [128, KC, 64, 2]),
                            op=ALU.mult)
                        nc.vector.tensor_tensor(
                            out=oh3p[:, :, 2, :, :],
                            in0=oh3p[:, :, 0, :, :],
                            in1=md_g[:, :, 4, None, :].to_broadcast(
                                [128, KC, 64, 2]),
                            op=ALU.mult)
                        # z = [x, x*u, x*u2]; z0 on scalar, z1/z2 on DVE
                        zst = data.tile([128, KC, 3, 64], BF16, tag="zst")
                        zstp = zst.rearrange("p k a (r t) -> p k a r t", t=2)
                        for half in range(2):
                            ks = slice(half * (KC // 2), (half + 1) * (KC // 2))
                            h0 = half * 64
                            xsv = xs4[:, half * 2:(half + 1) * 2, :, :]
                            xsv = xsv.rearrange(
                                "p q (c j) e -> p q c j e", j=GB)[:, :, :, j,
                                                                  h0:h0 + 64]
                            nc.scalar.activation(
                                zst[:, ks, 0, :],
                                xsv.rearrange("p q c e -> p (q c) e"), AF.Copy)
                            xsp = xsv.rearrange(
                                "p q c (r t) -> p (q c) r t", t=2)
                            for ai, mi in ((1, 1), (2, 2)):
                                nc.vector.tensor_tensor(
                                    out=zstp[:, ks, ai, :, :],
                                    in0=xsp[:],
                                    in1=md_g[:, ks, mi, None, :].to_broadcast(
                                        [128, KC // 2, 32, 2]),
                                    op=ALU.mult)

                        s_ps = [ps_s.tile([128, 3 * C], FP32, tag=f"s{bb}",
                                          name=f"sps{bb}")
                                for bb in range(3)]
                        for t in range(KC):
                            z_t = zst[:, t, :, :].rearrange("p a c -> p (a c)")
                            for bb in range(3):
                                nc.tensor.matmul(
                                    s_ps[bb][:], oh3[:, t, bb, :], z_t,
                                    start=(t == 0), stop=(t == KC - 1))

                        s_sb = sfin.tile([128, 640], BF16, tag="ssb")
                        for bb in range(3):
                            nc.scalar.activation(
                                s_sb[:, bb * 192:(bb + 1) * 192],
                                s_ps[bb][:], AF.Copy)
                        nc.vector.memset(s_sb[:, 576:640], 0.0)

                        o_ps = ps_o.tile([128, C], FP32, tag="ops")
                        sst_sb = sfin.tile([128, N_WCHUNK, 128], BF16,
                                           tag="sst")
                        for jj in range(N_WCHUNK):
                            st_ps = ps_t.tile([128, 128], BF16, tag="stps")
                            nc.tensor.transpose(
                                st_ps[:], s_sb[:, jj * 128:(jj + 1) * 128],
                                ident[:])
                            nc.scalar.activation(sst_sb[:, jj, :], st_ps[:],
                                                 AF.Copy)
                        for jj in range(N_WCHUNK):
                            nc.tensor.matmul(o_ps[:], sst_sb[:, jj, :],
                                             w_t[:, jj, :],
                                             start=(jj == 0),
                                             stop=(jj == N_WCHUNK - 1))
                        r = g % N_HRAW
                        kpos = (g % SB) // N_HRAW
                        if g % SB == 0:
                            layer.o_tiles = [
                                oacc.tile([128, K_SC, C], FP32,
                                          tag=f"oacc{rr}", name=f"ot{rr}")
                                for rr in range(N_HRAW)]
                        nc.scalar.activation(layer.o_tiles[r][:, kpos, :],
                                             o_ps[:], AF.Copy)
                        if g % SB == SB - 1:
                            B = g // SB
                            for rr in range(N_HRAW):
                                nc.gpsimd.dma_scatter_add(
                                    out_ap=h_raw[rr][:],
                                    in_ap=layer.o_tiles[rr][:],
                                    idxs_ap=scat_all[:, (B * N_HRAW + rr) * K_SC * 8:
                                                     (B * N_HRAW + rr + 1) * K_SC * 8],
                                    num_idxs=K_SC * 128,
                                    num_idxs_reg=K_SC * 128,
                                    elem_size=C,
                                    queue_num=rr)

                # ---- pass B ----
                TB = 7
                assert NT % TB == 0
                for blk in range(NT // TB):
                    rows = slice(blk * TB * 128, (blk + 1) * TB * 128)

                    def bview(buf):
                        return buf[rows, :].rearrange("(t p) c -> p t c",
                                                      p=128)

                    ha = pb.tile([128, TB, C], FP32, tag="ha")
                    nc.sync.dma_start(ha[:], bview(h_raw[0]))
                    hb2 = pb.tile([128, TB, C], FP32, tag="hb2")
                    nc.sync.dma_start(hb2[:], bview(h_raw[1]))
                    xo_t = pb.tile([128, TB, C], BF16, tag="xo")
                    nc.sync.dma_start(xo_t[:], bview(xo_src))
                    r_ps = ps_s.tile([128, TB, C], FP32, tag="rps")
                    for jj in range(TB):
                        xoT_ps = ps_t.tile([64, 128], BF16, tag="stps")
                        nc.tensor.transpose(xoT_ps[:], xo_t[:, jj, :],
                                            ident[:])
                        xoT_sb = pb.tile([64, 128], BF16, tag="xoTsb")
                        nc.scalar.activation(xoT_sb[:], xoT_ps[:], AF.Copy)
                        nc.tensor.matmul(r_ps[:, jj, :], xoT_sb[:], root_t[:],
                                         start=True, stop=True)
                    s1 = pb.tile([128, TB, C], FP32, tag="s1")
                    nc.gpsimd.tensor_tensor(out=s1[:], in0=ha[:], in1=hb2[:],
                                            op=ALU.add)
                    s4 = pb.tile([128, TB, C], FP32, tag="s4")
                    nc.vector.tensor_tensor(out=s4[:], in0=s1[:], in1=r_ps[:],
                                            op=ALU.add)
                    h_t = pb.tile([128, TB, C], out_dtype, tag="ht")
                    nc.scalar.activation(h_t[:], s4[:], AF.Relu)
                    nc.sync.dma_start(bview(out_dram), h_t[:])

            if _os.environ.get('K2_L1'):
                layer(t_xsrc_bf, w1_t, root1_t, bias1_t, t_out, FP32,
                      h_raw_l[0], xown_int)
            else:
                layer(t_xsrc_bf, w1_t, root1_t, bias1_t, h_own_bf, BF16,
                      h_raw_l[0], xown_int)
                if not _os.environ.get('K2_NOCOLL'):
                    nc.gpsimd.collective_compute(
                        "AllGather", ALU.bypass,
                        replica_groups=[list(range(N_CORES))],
                        ins=[h_own_bf.opt()], outs=[t_hfull_bf])
                layer(t_hfull_bf, w2_t, root2_t, bias2_t, t_out, FP32,
                      h_raw_l[1], h_own_bf)

    nc.compile()
    return nc


_CACHE = {}


def _get_program(NG_pad, NT_, n_src_rows):
    key = (NG_pad, NT_, n_src_rows)
    if key not in _CACHE:
        _CACHE[key] = build_program(NG_pad, NT_, n_src_rows)
    return _CACHE[key]


# ---------------------------------------------------------------------------
# Cached PJRT runner: jitted executable built once; static inputs stay
# device-resident; donated output buffers ping-pong from the previous call.
# ---------------------------------------------------------------------------

_DYNAMIC_INPUTS = ("xown",)


class _Runner:
    def __init__(self, nc, n_cores, dynamic_names=_DYNAMIC_INPUTS):
        import jax
        from jax.experimental.shard_map import shard_map
        from jax.sharding import Mesh, PartitionSpec, NamedSharding
        from concourse import bass2jax

        bass2jax.install_neuronx_cc_hook()
        self.jax = jax
        self.nc = nc
        self.n_cores = n_cores
        partition_name = (nc.partition_id_tensor.name
                          if nc.partition_id_tensor else None)
        in_names, out_names, out_avals, zero_outs = [], [], [], []
        for alloc in nc.m.functions[0].allocations:
            if not isinstance(alloc, mybir.MemoryLocationSet):
                continue
            name = alloc.memorylocations[0].name
            if alloc.kind == "ExternalInput":
                if name != partition_name:
                    in_names.append(name)
            elif alloc.kind == "ExternalOutput":
                shape = tuple(alloc.tensor_shape)
                dtype = mybir.dt.np(alloc.dtype)
                out_avals.append(jax.core.ShapedArray(shape, dtype))
                out_names.append(name)
                zero_outs.append(np.zeros(shape, dtype))
        assert nc.dbg_addr is None or not nc.dbg_callbacks
        self.dynamic_names = tuple(n for n in dynamic_names if n in in_names)
        self.in_names = in_names
        self.out_names = out_names
        self.out_avals = out_avals
        self.zero_outs = zero_outs
        n_params = len(in_names)
        n_outs = len(out_names)
        in_names_all = in_names + out_names
        if partition_name is not None:
            in_names_all.append(partition_name)
        donate = tuple(range(n_params, n_params + n_outs))

        def _body(*args):
            operands = list(args)
            if partition_name is not None:
                operands.append(bass2jax.partition_id_tensor())
            outs = bass2jax._bass_exec_p.bind(
                *operands,
                out_avals=tuple(out_avals),
                in_names=tuple(in_names_all),
                out_names=tuple(out_names),
                lowering_input_output_aliases=(),
                sim_require_finite=True,
                sim_require_nnan=True,
                nc=nc,
            )
            return tuple(outs)

        devices = jax.devices()[:n_cores]
        assert len(devices) == n_cores
        mesh = Mesh(np.asarray(devices), ("core",))
        self.sharding = NamedSharding(mesh, PartitionSpec("core"))
        in_specs = (PartitionSpec("core"),) * (n_params + n_outs)
        out_specs = (PartitionSpec("core"),) * n_outs
        self.fn = jax.jit(
            shard_map(_body, mesh=mesh, in_specs=in_specs,
                      out_specs=out_specs, check_rep=False),
            donate_argnums=donate, keep_unused=True)
        self.static_arrs = {}   # name -> device array
        self.prev_outs = None   # device arrays to donate as output buffers

    def _concat(self, in_maps, name):
        return np.concatenate(
            [np.asarray(in_maps[c][name]) for c in range(self.n_cores)], axis=0)

    def stage_static(self, in_maps):
        for name in self.in_names:
            if name in self.dynamic_names or name in self.static_arrs:
                continue
            darr = self.jax.device_put(self._concat(in_maps, name),
                                       self.sharding)
            darr.block_until_ready()
            self.static_arrs[name] = darr

    def run(self, in_maps):
        jax = self.jax
        t0 = _time.time()
        dyn = {}
        for name in self.dynamic_names:
            darr = jax.device_put(self._concat(in_maps, name), self.sharding)
            dyn[name] = darr
        for d in dyn.values():
            d.block_until_ready()
        t1 = _time.time()
        args = [dyn[n] if n in self.dynamic_names else self.static_arrs[n]
                for n in self.in_names]
        if self.prev_outs is None:
            outs_bufs = [
                jax.device_put(
                    np.zeros((self.n_cores * z.shape[0], *z.shape[1:]), z.dtype),
                    self.sharding)
                for z in self.zero_outs]
        else:
            outs_bufs = self.prev_outs
        t2 = _time.time()
        outs = self.fn(*args, *outs_bufs)
        jax.block_until_ready(outs)
        t3 = _time.time()
        self.prev_outs = list(outs)
        results = [np.asarray(o) for o in outs]
        t4 = _time.time()
        self.stage_seconds = t1 - t0
        self.exec_seconds = t3 - t2
        self.fetch_seconds = t4 - t3
        self.total_seconds = t4 - t0
        return {name: results[i].reshape(self.n_cores, *self.out_avals[i].shape)
                for i, name in enumerate(self.out_names)}

    def time_marginal_exec(self, in_maps, iters=10):
        """Per-execution device time via pipelined launches: executions are
        chained through the donated output buffer, so they serialize on
        device; (T_iters - T_1)/(iters-1) cancels the host-tunnel sync RTT.
        Returns (seconds_per_exec, results_dict) with results from the last
        execution (identical inputs -> identical output each run)."""
        jax = self.jax
        assert self.prev_outs is not None, "call run() first to warm"
        dyn = {name: jax.device_put(self._concat(in_maps, name), self.sharding)
               for name in self.dynamic_names}
        for d in dyn.values():
            d.block_until_ready()
        args = [dyn[n] if n in self.dynamic_names else self.static_arrs[n]
                for n in self.in_names]
        outs = self.prev_outs

        def timed_chain(n, outs):
            t0 = _time.time()
            for _ in range(n):
                outs = self.fn(*args, *outs)
            jax.block_until_ready(outs)
            return _time.time() - t0, outs

        # two chain lengths; slope cancels the tunnel sync RTT
        n_lo, n_hi = iters, iters * 5
        t_lo, outs = timed_chain(n_lo, outs)
        t_hi, outs = timed_chain(n_hi, outs)
        self.prev_outs = list(outs)
        per_exec = max(0.0, (t_hi - t_lo) / (n_hi - n_lo))
        self.marginal_detail = (n_lo, t_lo, n_hi, t_hi)
        results = [np.asarray(o) for o in outs]
        res = {name: results[i].reshape(self.n_cores, *self.out_avals[i].shape)
               for i, name in enumerate(self.out_names)}
        return per_exec, res



_RUNNERS = {}


def _get_runner(nc, key):
    if key not in _RUNNERS:
        _RUNNERS[key] = _Runner(nc, N_CORES)
    return _RUNNERS[key]


def _assemble_out(res, NPC):
    out_pc = res['out']
    out = np.zeros((N_NODES, C), np.float32)
    for c in range(N_CORES):
        lo = c * NPC
        n_real = max(0, min(N_NODES - lo, NPC))
        if n_real > 0:
            out[lo:lo + n_real] = out_pc[c][:n_real]
    return out


def measure_exec(x, edge_index, pseudo, W1, root1, b1, W2, root2, b2,
                 iters=10):
    in_maps, meta = prep(edge_index, pseudo, x, W1, root1, b1,
                         W2, root2, b2, N_NODES, NT)
    key = (meta['NG'], meta['NT'], meta['n_src_rows'])
    nc = _get_program(*key)
    runner = _get_runner(nc, key)
    runner.stage_static(in_maps)
    if runner.prev_outs is None:
        runner.run(in_maps)
    per_exec, res = runner.time_marginal_exec(in_maps, iters=iters)
    return per_exec, _assemble_out(res, meta['NPC'])


def kernel(x, edge_index, pseudo, W1, root1, b1, W2, root2, b2):
    in_maps, meta = prep(edge_index, pseudo, x, W1, root1, b1,
                         W2, root2, b2, N_NODES, NT)
    key = (meta['NG'], meta['NT'], meta['n_src_rows'])
    nc = _get_program(*key)
    runner = _get_runner(nc, key)
    runner.stage_static(in_maps)
    _t0 = _time.time()
    res = runner.run(in_maps)
    kernel.last_total_seconds = _time.time() - _t0
    kernel.last_exec_seconds = runner.exec_seconds
    kernel.last_stage_seconds = runner.stage_seconds
    kernel.last_fetch_seconds = runner.fetch_seconds
    return _assemble_out(res, meta['NPC'])
